# revision 3
# baseline (speedup 1.0000x reference)
"""Trainium2 Bass kernel for nn_ADSA_31061203484966 (channel-attention dense
transformer block). Pure data-parallel over batch B=8 across 8 NeuronCores.

Self-contained: hardcodes shapes; host-side numpy folds BN into conv
weights, folds depthwise+pointwise+attention-scale into dense per-tap
matrices, and the device kernel does all convs as shifted f32r matmuls
over a zero-padded [C, 4360] layout.
"""
import sys

for _p in ("/opt/trn_rl_repo", "/root/.axon_site/_ro/trn_rl_repo"):
    if _p not in sys.path:
        sys.path.append(_p)

import numpy as np
import concourse.bass as bass
import concourse.tile as tile
from concourse import bacc, mybir
from concourse.bass_utils import run_bass_kernel_spmd

f32 = mybir.dt.float32
f32r = mybir.dt.float32r
AF = mybir.ActivationFunctionType
OP = mybir.AluOpType

B, C, H, W = 8, 256, 64, 64
NH, HD = 4, 64
N = H * W                    # 4096
EPS = 1e-5
PADLEN = 4360                # 66*66 guarded padded row-major layout (+4 slack)
# image pixel (r, c) lives at column 68 + 66*r + c
TAPS = [(ky - 1, kx - 1) for ky in range(3) for kx in range(3)]

_CACHE = {}


def _pad_off(row, dx=0):
    return 68 + 66 * row + dx


def _pad_dst(tl, nt):
    """Strided write AP covering compact rows [8nt, 8nt+8) of a padded tile."""
    off = _pad_off(8 * nt)
    return tl[:, off:off + 528].rearrange("p (r c) -> p r c", c=66)[:, :, 0:64]


def _pad_rhs(tl, nt, dy, dx):
    """Conv rhs AP: 8 rows x 64 cols shifted by tap (dy, dx)."""
    off = _pad_off(8 * nt + dy, dx)
    return tl[:, off:off + 528].rearrange("p (r c) -> p r c", c=66)[:, :, 0:64]


def _zero_pads(nc, tl, zeros):
    """Zero the pad region of a [128, PADLEN] tile (dtype-safe via copies)."""
    nc.vector.tensor_copy(tl[:, 0:68], zeros[:, 0:68])
    nc.vector.tensor_copy(
        tl[:, 132:132 + 64 * 66].rearrange("p (r c) -> p r c", c=66)[:, :, 0:2],
        zeros[:, 0:128].rearrange("p (r c) -> p r c", c=2))
    nc.vector.tensor_copy(tl[:, 4292:4360], zeros[:, 0:68])


def _build():
    nc = bacc.Bacc("TRN2", target_bir_lowering=False, debug=False, num_devices=8)

    x_d = nc.dram_tensor("x", [C, N], f32, kind="ExternalInput").ap()
    qkvw_d = nc.dram_tensor("qkvw", [3, 9, 2, 2, 128, 128], f32, kind="ExternalInput").ap()
    dsw_d = nc.dram_tensor("dsw", [4, 9, 2, 128, 256], f32, kind="ExternalInput").ap()
    fusew_d = nc.dram_tensor("fusew", [9, 4, 2, 128, 128], f32, kind="ExternalInput").ap()
    mlpw_d = nc.dram_tensor("mlpw", [9, 2, 2, 128, 128], f32, kind="ExternalInput").ap()
    consts_d = nc.dram_tensor("consts", [2, 128, 16], f32, kind="ExternalInput").ap()
    ident_d = nc.dram_tensor("ident", [128, 128], f32, kind="ExternalInput").ap()
    out_d = nc.dram_tensor("out", [C, N], f32, kind="ExternalOutput").ap()

    with tile.TileContext(nc) as tc:
        with tc.tile_pool(name="persist", bufs=1) as persist, \
             tc.tile_pool(name="psA", bufs=3, space="PSUM") as psA, \
             tc.tile_pool(name="psB", bufs=2, space="PSUM") as psB:

            zeros = persist.tile([128, 128], f32, name="zeros")
            nc.vector.memset(zeros[:], 0.0)
            ones = persist.tile([128, 128], f32, name="ones")
            nc.vector.memset(ones[:], 1.0)
            ident = persist.tile([128, 128], f32, name="ident")
            nc.sync.dma_start(ident[:], ident_d[:])
            consts = [persist.tile([128, 16], f32, name=f"consts{m}") for m in range(2)]
            for m in range(2):
                nc.sync.dma_start(consts[m][:], consts_d[m])

            v_pad = [persist.tile([128, PADLEN], f32r, name=f"v_pad{m}") for m in range(2)]
            for m in range(2):
                _zero_pads(nc, v_pad[m], zeros)

            # blockdiag attention-weight chunks, one per (variant, chunk)
            wblk = {}
            for xx in range(4):
                for ch in range(2):
                    t_ = persist.tile([128, 128], f32r, name=f"wblk{xx}_{ch}")
                    nc.vector.tensor_copy(t_[:], zeros[:])
                    wblk[(xx, ch)] = t_
            ab_sb = [persist.tile([128, 65], f32, name=f"ab{i}") for i in range(8)]
            tmp_wt = [persist.tile([128, 64], f32, name=f"tmpwt{h}") for h in range(4)]
            nm_rq = [persist.tile([128, 1], f32, name=f"nmrq{h}") for h in range(4)]

            with tc.tile_pool(name="pqt", bufs=1) as pqt:
                qT = pqt.tile([128, 32 * 260 + 64], f32, name="qT")
                kT = pqt.tile([128, 32 * 260 + 64], f32, name="kT")
                # ones columns (65th col of each head slot in every chunk)
                for T_all in (qT, kT):
                    nc.vector.tensor_copy(
                        T_all[:, 64:64 + 65 * 128].rearrange(
                            "p (a b) -> p a b", b=65)[:, :, 0:1],
                        ones[:, 0:128].rearrange("p (a b) -> p a b", b=1))

                with tc.tile_pool(name="px", bufs=1) as px, \
                     tc.tile_pool(name="wq", bufs=2) as wq, \
                     tc.tile_pool(name="stq", bufs=4) as stq:
                    x_pad = [px.tile([128, PADLEN], f32r, name=f"x_pad{m}") for m in range(2)]
                    for m in range(2):
                        _zero_pads(nc, x_pad[m], zeros)
                        nc.sync.dma_start(
                            x_pad[m][:, 68:68 + 64 * 66].rearrange(
                                "p (r c) -> p r c", c=66)[:, :, 0:64],
                            x_d[128 * m:128 * m + 128, :].rearrange(
                                "p (r c) -> p r c", c=64).bitcast(f32r))

                    # ---- qkv convs (j: 0=q, 1=k, 2=v) ----
                    for j in range(3):
                        for m in range(2):
                            wts = {}
                            for t in range(9):
                                for ic in range(2):
                                    w_ = wq.tile([128, 128], f32r, tag=f"w{t}_{ic}",
                                                 name=f"qkvw{j}{m}{t}{ic}")
                                    nc.sync.dma_start(w_[:], qkvw_d[j, t, ic, m].bitcast(f32r))
                                    wts[(t, ic)] = w_
                            for nt in range(8):
                                ps = psA.tile([128, 512], f32, tag="conv",
                                              name=f"cv{j}{m}{nt}")
                                mm = 0
                                for t, (dy, dx) in enumerate(TAPS):
                                    for ic in range(2):
                                        nc.tensor.matmul(
                                            ps[:], wts[(t, ic)][:],
                                            _pad_rhs(x_pad[ic], nt, dy, dx),
                                            start=(mm == 0), stop=(mm == 17))
                                        mm += 1
                                if j == 2:       # v -> padded buffer directly
                                    nc.scalar.activation(
                                        _pad_dst(v_pad[m], nt),
                                        ps[:].rearrange("p (r c) -> p r c", c=64),
                                        AF.Identity, bias=consts[m][:, 2:3], scale=1.0)
                                else:            # q, k -> stage -> transpose -> qT/kT
                                    T_all = qT if j == 0 else kT
                                    stg = stq.tile([128, 512], f32, tag="stage",
                                                   name=f"stg{j}{m}{nt}")
                                    nc.scalar.activation(
                                        stg[:], ps[:], AF.Identity,
                                        bias=consts[m][:, j:j + 1], scale=1.0)
                                    for bb in range(4):
                                        pst = psA.tile([128, 128], f32, tag="tr",
                                                       name=f"tr{j}{m}{nt}{bb}")
                                        nc.tensor.transpose(
                                            pst[:], stg[:, 128 * bb:128 * bb + 128],
                                            ident[:])
                                        base = 260 * (4 * nt + bb) + 130 * m
                                        nc.vector.tensor_copy(
                                            T_all[:, base:base + 130].rearrange(
                                                "p (h d) -> p h d", d=65)[:, :, 0:64],
                                            pst[:].rearrange("p (h d) -> p h d", d=64))

                # ---- margins: per head, A = kT'.qT (S^T + margins), B = qT'.kT ----
                for h in range(4):
                    for ab in range(2):
                        lhsT_src, rhs_src = (kT, qT) if ab == 0 else (qT, kT)
                        ps = psA.tile([128, 512], f32, tag="conv", name=f"mg{h}{ab}")
                        for c in range(32):
                            base = 260 * c + 65 * h
                            nc.tensor.matmul(
                                ps[0:65, 0:65],
                                lhsT_src[:, base:base + 65],
                                rhs_src[:, base:base + 65],
                                start=(c == 0), stop=(c == 31))
                        dst = ab_sb[2 * h + ab]
                        nc.vector.tensor_copy(dst[0:65, :], ps[0:65, 0:65])

                # ---- W formation into blockdiag slots ----
                for h in range(4):
                    a_sb, b_sb = ab_sb[2 * h], ab_sb[2 * h + 1]
                    ch, p0 = h // 2, 64 * (h % 2)
                    sl = slice(p0, p0 + 64)
                    # W_aa = S  (from B)
                    nc.vector.tensor_copy(wblk[(0, ch)][sl, sl], b_sb[0:64, 0:64])
                    # W_ai = R_q - S
                    nc.vector.tensor_scalar(
                        wblk[(2, ch)][sl, sl], b_sb[0:64, 0:64],
                        -1.0, b_sb[0:64, 64:65], OP.mult, OP.add)
                    # W_ia^T = R_k - S^T (from A), then transpose
                    nc.vector.tensor_scalar(
                        tmp_wt[h][0:64, 0:64], a_sb[0:64, 0:64],
                        -1.0, a_sb[0:64, 64:65], OP.mult, OP.add)
                    psw = psA.tile([128, 128], f32, tag="tr", name=f"psw{h}")
                    nc.tensor.transpose(psw[0:64, 0:64], tmp_wt[h][0:64, 0:64],
                                        ident[0:64, 0:64])
                    nc.vector.tensor_copy(wblk[(3, ch)][sl, sl], psw[0:64, 0:64])
                    # W_ii = (N - R_q) - W_ia
                    nc.vector.tensor_scalar(
                        nm_rq[h][0:64, 0:1], b_sb[0:64, 64:65],
                        -1.0, float(N), OP.mult, OP.add)
                    nc.vector.tensor_scalar(
                        wblk[(1, ch)][sl, sl], psw[0:64, 0:64],
                        -1.0, nm_rq[h][0:64, 0:1], OP.mult, OP.add)

            with tc.tile_pool(name="pcat", bufs=1) as pcat:
                cat_pad = [pcat.tile([128, PADLEN], f32r, name=f"cat_pad{i}")
                           for i in range(4)]
                for i in range(4):
                    _zero_pads(nc, cat_pad[i], zeros)

                with tc.tile_pool(name="pcw", bufs=1) as pcw, \
                     tc.tile_pool(name="wd", bufs=4) as wd:
                    cw = pcw.tile([128, 9216], f32r, name="cw")
                    # CW[pair, t, ic] = sum_xx blockdiag(W_xx)^T.T @ dsT[xx, t][ic]
                    for pair in range(2):
                        for t in range(9):
                            for ic in range(2):
                                ps = psB.tile([128, 256], f32, tag="mg",
                                              name=f"cwp{pair}{t}{ic}")
                                for xi in range(2):
                                    xx = 2 * pair + xi
                                    w_ = wd.tile([128, 256], f32r, tag="dsw",
                                                 name=f"dsw{xx}{t}{ic}")
                                    nc.sync.dma_start(w_[:], dsw_d[xx, t, ic].bitcast(f32r))
                                    nc.tensor.matmul(ps[:], wblk[(xx, ic)][:], w_[:],
                                                     start=(xi == 0), stop=(xi == 1))
                                col = ((pair * 9 + t) * 2 + ic) * 256
                                nc.vector.tensor_copy(cw[:, col:col + 256], ps[:])

                    # ---- cat convs over v_pad ----
                    for pair in range(2):
                        for oc in range(2):
                            for nt in range(8):
                                ps = psA.tile([128, 512], f32, tag="conv",
                                              name=f"cc{pair}{oc}{nt}")
                                mm = 0
                                for t, (dy, dx) in enumerate(TAPS):
                                    for ic in range(2):
                                        col = ((pair * 9 + t) * 2 + ic) * 256 + 128 * oc
                                        nc.tensor.matmul(
                                            ps[:], cw[:, col:col + 128],
                                            _pad_rhs(v_pad[ic], nt, dy, dx),
                                            start=(mm == 0), stop=(mm == 17))
                                        mm += 1
                                nc.scalar.activation(
                                    _pad_dst(cat_pad[2 * pair + oc], nt),
                                    ps[:].rearrange("p (r c) -> p r c", c=64),
                                    AF.Identity, bias=consts[oc][:, 3 + pair:4 + pair],
                                    scale=1.0)

                with tc.tile_pool(name="py2", bufs=1) as py2, \
                     tc.tile_pool(name="wf", bufs=2) as wf, \
                     tc.tile_pool(name="stf", bufs=4) as stf:
                    y2_pad = [py2.tile([128, PADLEN], f32r, name=f"y2_pad{m}")
                              for m in range(2)]
                    for m in range(2):
                        _zero_pads(nc, y2_pad[m], zeros)

                    # ---- fuse conv + gelu + residual(v) + norm affine ----
                    for oc in range(2):
                        fw = {}
                        for t in range(9):
                            for ic in range(4):
                                w_ = wf.tile([128, 128], f32r, tag=f"f{t}_{ic}",
                                             name=f"fw{oc}{t}{ic}")
                                nc.sync.dma_start(w_[:], fusew_d[t, ic, oc].bitcast(f32r))
                                fw[(t, ic)] = w_
                        for nt in range(8):
                            ps = psA.tile([128, 512], f32, tag="conv",
                                          name=f"fc{oc}{nt}")
                            mm = 0
                            for t, (dy, dx) in enumerate(TAPS):
                                for ic in range(4):
                                    nc.tensor.matmul(
                                        ps[:], fw[(t, ic)][:],
                                        _pad_rhs(cat_pad[ic], nt, dy, dx),
                                        start=(mm == 0), stop=(mm == 35))
                                    mm += 1
                            g1 = stf.tile([128, 512], f32, tag="g1", name=f"g1{oc}{nt}")
                            nc.scalar.activation(g1[:], ps[:], AF.Gelu_apprx_tanh,
                                                 bias=consts[oc][:, 5:6], scale=1.0)
                            g2 = stf.tile([128, 512], f32, tag="g2", name=f"g2{oc}{nt}")
                            nc.vector.tensor_tensor(
                                g2[:].rearrange("p (r c) -> p r c", c=64),
                                g1[:].rearrange("p (r c) -> p r c", c=64),
                                _pad_rhs(v_pad[oc], nt, 0, 0).bitcast(f32), OP.add)
                            nc.vector.tensor_scalar(
                                _pad_dst(y2_pad[oc], nt),
                                g2[:].rearrange("p (r c) -> p r c", c=64),
                                consts[oc][:, 6:7], consts[oc][:, 7:8],
                                OP.mult, OP.add)

                    # ---- mlp conv + gelu + residual(y2) -> out ----
                    for oc in range(2):
                        mw = {}
                        for t in range(9):
                            for ic in range(2):
                                w_ = wf.tile([128, 128], f32r, tag=f"f{t}_{ic}",
                                             name=f"mw{oc}{t}{ic}")
                                nc.sync.dma_start(w_[:], mlpw_d[t, ic, oc].bitcast(f32r))
                                mw[(t, ic)] = w_
                        for nt in range(8):
                            ps = psA.tile([128, 512], f32, tag="conv",
                                          name=f"mc{oc}{nt}")
                            mm = 0
                            for t, (dy, dx) in enumerate(TAPS):
                                for ic in range(2):
                                    nc.tensor.matmul(
                                        ps[:], mw[(t, ic)][:],
                                        _pad_rhs(y2_pad[ic], nt, dy, dx),
                                        start=(mm == 0), stop=(mm == 17))
                                    mm += 1
                            g1 = stf.tile([128, 512], f32, tag="g1", name=f"mg1{oc}{nt}")
                            nc.scalar.activation(g1[:], ps[:], AF.Gelu_apprx_tanh,
                                                 bias=consts[oc][:, 8:9], scale=1.0)
                            g3 = stf.tile([128, 512], f32, tag="g2", name=f"mo{oc}{nt}")
                            nc.vector.tensor_tensor(
                                g3[:].rearrange("p (r c) -> p r c", c=64),
                                g1[:].rearrange("p (r c) -> p r c", c=64),
                                _pad_rhs(y2_pad[oc], nt, 0, 0).bitcast(f32), OP.add)
                            nc.sync.dma_start(
                                out_d[128 * oc:128 * oc + 128, 512 * nt:512 * nt + 512],
                                g3[:])

    nc.compile()
    return nc


def _prep(inputs):
    def bn_fold(g, b, m, v):
        s = g.astype(np.float64) / np.sqrt(v.astype(np.float64) + EPS)
        return s, b.astype(np.float64) - m.astype(np.float64) * s

    scale = C ** (-0.5)
    s_qkv, b_qkv = bn_fold(inputs['qkv_g'], inputs['qkv_b'], inputs['qkv_m'], inputs['qkv_v'])
    qkv_w = inputs['qkv_w'].astype(np.float64)
    # [j, tap, i, o]
    qkv_wT = (qkv_w * s_qkv[:, :, None, None, None]).transpose(0, 3, 4, 2, 1).reshape(3, 9, C, C)
    qkvw = qkv_wT.reshape(3, 9, 2, 128, 2, 128).transpose(0, 1, 2, 4, 3, 5).astype(np.float32)
    qkvw = np.ascontiguousarray(qkvw)  # [3, 9, ic, oc, 128, 128]

    s_ds, b_ds = bn_fold(inputs['ds_g'], inputs['ds_b'], inputs['ds_m'], inputs['ds_v'])
    pw = inputs['pw_w'].astype(np.float64)[:, :, :, 0, 0]              # [4, o, i]
    dw = inputs['dw_w'].astype(np.float64)[:, :, 0, :, :].reshape(4, C, 9)  # [4, i, tap]
    # dsT[xx, tap, i, o] = pw[xx,o,i] * dw[xx,i,tap] * s_ds[xx,o] * scale
    dsT = (pw.transpose(0, 2, 1)[:, None, :, :] * dw.transpose(0, 2, 1)[:, :, :, None]
           * s_ds[:, None, None, :]) * scale
    dsw = np.ascontiguousarray(dsT.reshape(4, 9, 2, 128, C).astype(np.float32))

    s_f, b_f = bn_fold(inputs['fuse_g'], inputs['fuse_b'], inputs['fuse_m'], inputs['fuse_v'])
    fuse_wT = (inputs['fuse_w'].astype(np.float64) * s_f[:, None, None, None]
               ).transpose(2, 3, 1, 0).reshape(9, 2 * C, C)
    fusew = np.ascontiguousarray(
        fuse_wT.reshape(9, 4, 128, 2, 128).transpose(0, 1, 3, 2, 4).astype(np.float32))

    s_n, t_n = bn_fold(inputs['norm_g'], inputs['norm_b'], inputs['norm_m'], inputs['norm_v'])
    s_m, b_m = bn_fold(inputs['mlp_g'], inputs['mlp_b'], inputs['mlp_m'], inputs['mlp_v'])
    mlp_wT = (inputs['mlp_w'].astype(np.float64) * s_m[:, None, None, None]
              ).transpose(2, 3, 1, 0).reshape(9, C, C)
    mlpw = np.ascontiguousarray(
        mlp_wT.reshape(9, 2, 128, 2, 128).transpose(0, 1, 3, 2, 4).astype(np.float32))

    consts = np.zeros((2, 128, 16), np.float64)
    cols = [b_qkv[0], b_qkv[1], b_qkv[2],
            b_ds[0] + b_ds[1], b_ds[2] + b_ds[3],
            b_f, s_n, t_n, b_m]
    for ci, v in enumerate(cols):
        consts[0, :, ci] = v[0:128]
        consts[1, :, ci] = v[128:256]
    consts = consts.astype(np.float32)

    ident = np.eye(128, dtype=np.float32)
    return qkvw, dsw, fusew, mlpw, consts, ident


def kernel(**inputs):
    inputs = {k: np.asarray(v) for k, v in inputs.items()}
    if "nc" not in _CACHE:
        _CACHE["nc"] = _build()
    nc = _CACHE["nc"]

    qkvw, dsw, fusew, mlpw, consts, ident = _prep(inputs)
    shared = {"qkvw": qkvw, "dsw": dsw, "fusew": fusew, "mlpw": mlpw,
              "consts": consts, "ident": ident}
    x = inputs['x'].astype(np.float32).reshape(B, C, N)
    in_maps = [{"x": np.ascontiguousarray(x[b]), **shared} for b in range(B)]

    res = run_bass_kernel_spmd(nc, in_maps, core_ids=list(range(8)))
    out = np.stack([res.results[b]["out"] for b in range(B)])
    return out.reshape(B, C, H, W).astype(np.float32)


# revision 4
# speedup vs baseline: 1.0205x; 1.0205x over previous
"""Trainium2 Bass kernel for nn_ADSA_31061203484966 (channel-attention dense
transformer block). Pure data-parallel over batch B=8 across 8 NeuronCores.

Self-contained: hardcodes shapes; host-side numpy folds BN into conv
weights, folds depthwise+pointwise+attention-scale into dense per-tap
matrices, and the device kernel does all convs as shifted f32r matmuls
over a zero-padded [C, 4360] layout.
"""
import sys

for _p in ("/opt/trn_rl_repo", "/root/.axon_site/_ro/trn_rl_repo"):
    if _p not in sys.path:
        sys.path.append(_p)

import numpy as np
import concourse.bass as bass
import concourse.tile as tile
from concourse import bacc, mybir
from concourse.bass_utils import run_bass_kernel_spmd

f32 = mybir.dt.float32
f32r = mybir.dt.float32r
AF = mybir.ActivationFunctionType
OP = mybir.AluOpType

B, C, H, W = 8, 256, 64, 64
NH, HD = 4, 64
N = H * W                    # 4096
EPS = 1e-5
PADLEN = 4360                # 66*66 guarded padded row-major layout (+4 slack)
# image pixel (r, c) lives at column 68 + 66*r + c
TAPS = [(ky - 1, kx - 1) for ky in range(3) for kx in range(3)]

_CACHE = {}


def _pad_off(row, dx=0):
    return 68 + 66 * row + dx


def _pad_dst(tl, nt):
    """Strided write AP covering compact rows [8nt, 8nt+8) of a padded tile."""
    off = _pad_off(8 * nt)
    return tl[:, off:off + 528].rearrange("p (r c) -> p r c", c=66)[:, :, 0:64]


def _pad_rhs(tl, nt, dy, dx):
    """Conv rhs AP: 8 rows x 64 cols shifted by tap (dy, dx)."""
    off = _pad_off(8 * nt + dy, dx)
    return tl[:, off:off + 528].rearrange("p (r c) -> p r c", c=66)[:, :, 0:64]


def _zero_pads(nc, tl, zeros):
    """Zero the pad region of a [128, PADLEN] tile (dtype-safe via copies)."""
    nc.vector.tensor_copy(tl[:, 0:68], zeros[:, 0:68])
    nc.vector.tensor_copy(
        tl[:, 132:132 + 64 * 66].rearrange("p (r c) -> p r c", c=66)[:, :, 0:2],
        zeros[:, 0:128].rearrange("p (r c) -> p r c", c=2))
    nc.vector.tensor_copy(tl[:, 4292:4360], zeros[:, 0:68])


def _conv_block(nc, psA, name, wts, srcs, n_in, evac):
    """Weight-stationary grouped conv: for each 4-tile block, loop weights
    outer and 4 output tiles inner so consecutive matmuls share lhsT.

    wts: dict (t, ic) -> weight AP; srcs: list of padded input tiles
    (len n_in); evac(nt, ps) emits the PSUM evacuation."""
    n_mm = 9 * n_in
    for ntb in range(2):
        pss = [psA.tile([128, 512], f32, tag="conv", name=f"{name}ps{ntb}_{i}")
               for i in range(4)]
        mm = 0
        for t, (dy, dx) in enumerate(TAPS):
            for ic in range(n_in):
                w_ = wts[(t, ic)]
                for ntl in range(4):
                    nt = 4 * ntb + ntl
                    nc.tensor.matmul(
                        pss[ntl][:], w_[:], _pad_rhs(srcs[ic], nt, dy, dx),
                        start=(mm == 0), stop=(mm == n_mm - 1))
                mm += 1
        for ntl in range(4):
            evac(4 * ntb + ntl, pss[ntl])


def _build():
    nc = bacc.Bacc("TRN2", target_bir_lowering=False, debug=False, num_devices=8)

    xp_d = nc.dram_tensor("xp", [2, 128, PADLEN], f32, kind="ExternalInput").ap()
    qkvw_d = nc.dram_tensor("qkvw", [3, 9, 2, 2, 128, 128], f32, kind="ExternalInput").ap()
    dsw_d = nc.dram_tensor("dsw", [4, 9, 2, 128, 256], f32, kind="ExternalInput").ap()
    fusew_d = nc.dram_tensor("fusew", [9, 4, 2, 128, 128], f32, kind="ExternalInput").ap()
    mlpw_d = nc.dram_tensor("mlpw", [9, 2, 2, 128, 128], f32, kind="ExternalInput").ap()
    consts_d = nc.dram_tensor("consts", [2, 128, 16], f32, kind="ExternalInput").ap()
    ident_d = nc.dram_tensor("ident", [128, 128], f32, kind="ExternalInput").ap()
    out_d = nc.dram_tensor("out", [C, N], f32, kind="ExternalOutput").ap()

    with tile.TileContext(nc) as tc:
        with tc.tile_pool(name="persist", bufs=1) as persist, \
             tc.tile_pool(name="psA", bufs=4, space="PSUM") as psA, \
             tc.tile_pool(name="psT", bufs=2, space="PSUM") as psT, \
             tc.tile_pool(name="psB", bufs=2, space="PSUM") as psB:

            zeros = persist.tile([128, 128], f32, name="zeros")
            nc.vector.memset(zeros[:], 0.0)
            ones = persist.tile([128, 128], f32, name="ones")
            nc.vector.memset(ones[:], 1.0)
            ident = persist.tile([128, 128], f32, name="ident")
            nc.sync.dma_start(ident[:], ident_d[:])
            consts = [persist.tile([128, 16], f32, name=f"consts{m}") for m in range(2)]
            for m in range(2):
                nc.sync.dma_start(consts[m][:], consts_d[m])

            v_pad = [persist.tile([128, PADLEN], f32r, name=f"v_pad{m}") for m in range(2)]
            for m in range(2):
                _zero_pads(nc, v_pad[m], zeros)

            # blockdiag attention-weight chunks, one per (variant, chunk)
            wblk = {}
            for xx in range(4):
                for ch in range(2):
                    t_ = persist.tile([128, 128], f32r, name=f"wblk{xx}_{ch}")
                    nc.vector.tensor_copy(t_[:], zeros[:])
                    wblk[(xx, ch)] = t_
            ab_sb = [persist.tile([128, 65], f32, name=f"ab{i}") for i in range(8)]
            tmp_wt = [persist.tile([128, 64], f32, name=f"tmpwt{h}") for h in range(4)]
            nm_rq = [persist.tile([128, 1], f32, name=f"nmrq{h}") for h in range(4)]

            with tc.tile_pool(name="pqt", bufs=1) as pqt:
                qT = pqt.tile([128, 32 * 260 + 64], f32, name="qT")
                kT = pqt.tile([128, 32 * 260 + 64], f32, name="kT")
                # ones columns (65th col of each head slot in every chunk)
                for T_all in (qT, kT):
                    nc.vector.tensor_copy(
                        T_all[:, 64:64 + 65 * 128].rearrange(
                            "p (a b) -> p a b", b=65)[:, :, 0:1],
                        ones[:, 0:128].rearrange("p (a b) -> p a b", b=1))

                with tc.tile_pool(name="px", bufs=1) as px, \
                     tc.tile_pool(name="wq", bufs=2) as wq, \
                     tc.tile_pool(name="stq", bufs=4) as stq:
                    x_pad = [px.tile([128, PADLEN], f32r, name=f"x_pad{m}") for m in range(2)]
                    for m in range(2):
                        nc.sync.dma_start(x_pad[m][:], xp_d[m].bitcast(f32r))

                    # ---- qkv convs (j: 0=q, 1=k, 2=v) ----
                    for j in range(3):
                        for m in range(2):
                            wts = {}
                            for t in range(9):
                                for ic in range(2):
                                    w_ = wq.tile([128, 128], f32r, tag=f"w{t}_{ic}",
                                                 name=f"qkvw{j}{m}{t}{ic}")
                                    nc.sync.dma_start(w_[:], qkvw_d[j, t, ic, m].bitcast(f32r))
                                    wts[(t, ic)] = w_

                            if j == 2:       # v -> padded buffer directly
                                def evac_v(nt, ps, m=m):
                                    nc.scalar.activation(
                                        _pad_dst(v_pad[m], nt),
                                        ps[:].rearrange("p (r c) -> p r c", c=64),
                                        AF.Identity, bias=consts[m][:, 2:3], scale=1.0)
                                _conv_block(nc, psA, f"v{m}", wts, x_pad, 2, evac_v)
                            else:            # q, k -> stage -> transpose -> qT/kT
                                T_all = qT if j == 0 else kT

                                def evac_qk(nt, ps, j=j, m=m, T_all=T_all):
                                    stg = stq.tile([128, 512], f32, tag="stage",
                                                   name=f"stg{j}{m}{nt}")
                                    nc.scalar.activation(
                                        stg[:], ps[:], AF.Identity,
                                        bias=consts[m][:, j:j + 1], scale=1.0)
                                    for bb in range(4):
                                        pst = psT.tile([128, 128], f32, tag="tr",
                                                       name=f"tr{j}{m}{nt}{bb}")
                                        nc.tensor.transpose(
                                            pst[:], stg[:, 128 * bb:128 * bb + 128],
                                            ident[:])
                                        base = 260 * (4 * nt + bb) + 130 * m
                                        nc.vector.tensor_copy(
                                            T_all[:, base:base + 130].rearrange(
                                                "p (h d) -> p h d", d=65)[:, :, 0:64],
                                            pst[:].rearrange("p (h d) -> p h d", d=64))
                                _conv_block(nc, psA, f"qk{j}{m}", wts, x_pad, 2, evac_qk)

                # ---- margins: per head, A = kT'.qT (S^T + margins), B = qT'.kT ----
                for h in range(4):
                    for ab in range(2):
                        lhsT_src, rhs_src = (kT, qT) if ab == 0 else (qT, kT)
                        ps = psA.tile([128, 512], f32, tag="conv", name=f"mg{h}{ab}")
                        for c in range(32):
                            base = 260 * c + 65 * h
                            nc.tensor.matmul(
                                ps[0:65, 0:65],
                                lhsT_src[:, base:base + 65],
                                rhs_src[:, base:base + 65],
                                start=(c == 0), stop=(c == 31))
                        dst = ab_sb[2 * h + ab]
                        nc.vector.tensor_copy(dst[0:65, :], ps[0:65, 0:65])

                # ---- W formation into blockdiag slots ----
                for h in range(4):
                    a_sb, b_sb = ab_sb[2 * h], ab_sb[2 * h + 1]
                    ch, p0 = h // 2, 64 * (h % 2)
                    sl = slice(p0, p0 + 64)
                    # W_aa = S  (from B)
                    nc.vector.tensor_copy(wblk[(0, ch)][sl, sl], b_sb[0:64, 0:64])
                    # W_ai = R_q - S
                    nc.vector.tensor_scalar(
                        wblk[(2, ch)][sl, sl], b_sb[0:64, 0:64],
                        -1.0, b_sb[0:64, 64:65], OP.mult, OP.add)
                    # W_ia^T = R_k - S^T (from A), then transpose
                    nc.vector.tensor_scalar(
                        tmp_wt[h][0:64, 0:64], a_sb[0:64, 0:64],
                        -1.0, a_sb[0:64, 64:65], OP.mult, OP.add)
                    psw = psT.tile([128, 128], f32, tag="tr", name=f"psw{h}")
                    nc.tensor.transpose(psw[0:64, 0:64], tmp_wt[h][0:64, 0:64],
                                        ident[0:64, 0:64])
                    nc.vector.tensor_copy(wblk[(3, ch)][sl, sl], psw[0:64, 0:64])
                    # W_ii = (N - R_q) - W_ia
                    nc.vector.tensor_scalar(
                        nm_rq[h][0:64, 0:1], b_sb[0:64, 64:65],
                        -1.0, float(N), OP.mult, OP.add)
                    nc.vector.tensor_scalar(
                        wblk[(1, ch)][sl, sl], psw[0:64, 0:64],
                        -1.0, nm_rq[h][0:64, 0:1], OP.mult, OP.add)

            with tc.tile_pool(name="pcat", bufs=1) as pcat:
                cat_pad = [pcat.tile([128, PADLEN], f32r, name=f"cat_pad{i}")
                           for i in range(4)]
                for i in range(4):
                    _zero_pads(nc, cat_pad[i], zeros)

                with tc.tile_pool(name="pcw", bufs=1) as pcw, \
                     tc.tile_pool(name="wd", bufs=4) as wd:
                    cw = pcw.tile([128, 9216], f32r, name="cw")
                    # CW[pair, t, ic] = sum_xx blockdiag(W_xx).T @ dsT[xx, t][ic]
                    for pair in range(2):
                        for t in range(9):
                            for ic in range(2):
                                ps = psB.tile([128, 256], f32, tag="mg",
                                              name=f"cwp{pair}{t}{ic}")
                                for xi in range(2):
                                    xx = 2 * pair + xi
                                    w_ = wd.tile([128, 256], f32r, tag="dsw",
                                                 name=f"dsw{xx}{t}{ic}")
                                    nc.sync.dma_start(w_[:], dsw_d[xx, t, ic].bitcast(f32r))
                                    nc.tensor.matmul(ps[:], wblk[(xx, ic)][:], w_[:],
                                                     start=(xi == 0), stop=(xi == 1))
                                col = ((pair * 9 + t) * 2 + ic) * 256
                                nc.vector.tensor_copy(cw[:, col:col + 256], ps[:])

                    # ---- cat convs over v_pad ----
                    for pair in range(2):
                        for oc in range(2):
                            cwts = {}
                            for t in range(9):
                                for ic in range(2):
                                    col = ((pair * 9 + t) * 2 + ic) * 256 + 128 * oc
                                    cwts[(t, ic)] = cw[:, col:col + 128]

                            def evac_cat(nt, ps, pair=pair, oc=oc):
                                nc.scalar.activation(
                                    _pad_dst(cat_pad[2 * pair + oc], nt),
                                    ps[:].rearrange("p (r c) -> p r c", c=64),
                                    AF.Identity,
                                    bias=consts[oc][:, 3 + pair:4 + pair], scale=1.0)
                            _conv_block(nc, psA, f"cc{pair}{oc}", cwts, v_pad, 2,
                                        evac_cat)

                with tc.tile_pool(name="py2", bufs=1) as py2, \
                     tc.tile_pool(name="wf", bufs=2) as wf, \
                     tc.tile_pool(name="stf", bufs=4) as stf:
                    y2_pad = [py2.tile([128, PADLEN], f32r, name=f"y2_pad{m}")
                              for m in range(2)]
                    for m in range(2):
                        _zero_pads(nc, y2_pad[m], zeros)

                    # ---- fuse conv + gelu + residual(v) + norm affine ----
                    for oc in range(2):
                        fwts = {}
                        for t in range(9):
                            for ic in range(4):
                                w_ = wf.tile([128, 128], f32r, tag=f"f{t}_{ic}",
                                             name=f"fw{oc}{t}{ic}")
                                nc.sync.dma_start(w_[:], fusew_d[t, ic, oc].bitcast(f32r))
                                fwts[(t, ic)] = w_

                        def evac_fuse(nt, ps, oc=oc):
                            g1 = stf.tile([128, 512], f32, tag="g1", name=f"g1{oc}{nt}")
                            nc.scalar.activation(g1[:], ps[:], AF.Gelu_apprx_tanh,
                                                 bias=consts[oc][:, 5:6], scale=1.0)
                            g2 = stf.tile([128, 512], f32, tag="g2", name=f"g2{oc}{nt}")
                            nc.vector.tensor_tensor(
                                g2[:].rearrange("p (r c) -> p r c", c=64),
                                g1[:].rearrange("p (r c) -> p r c", c=64),
                                _pad_rhs(v_pad[oc], nt, 0, 0).bitcast(f32), OP.add)
                            nc.vector.tensor_scalar(
                                _pad_dst(y2_pad[oc], nt),
                                g2[:].rearrange("p (r c) -> p r c", c=64),
                                consts[oc][:, 6:7], consts[oc][:, 7:8],
                                OP.mult, OP.add)
                        _conv_block(nc, psA, f"fc{oc}", fwts, cat_pad, 4, evac_fuse)

                    # ---- mlp conv + gelu + residual(y2) -> out ----
                    for oc in range(2):
                        mwts = {}
                        for t in range(9):
                            for ic in range(2):
                                w_ = wf.tile([128, 128], f32r, tag=f"f{t}_{ic}",
                                             name=f"mw{oc}{t}{ic}")
                                nc.sync.dma_start(w_[:], mlpw_d[t, ic, oc].bitcast(f32r))
                                mwts[(t, ic)] = w_

                        def evac_mlp(nt, ps, oc=oc):
                            g1 = stf.tile([128, 512], f32, tag="g1", name=f"mg1{oc}{nt}")
                            nc.scalar.activation(g1[:], ps[:], AF.Gelu_apprx_tanh,
                                                 bias=consts[oc][:, 8:9], scale=1.0)
                            g3 = stf.tile([128, 512], f32, tag="g2", name=f"mo{oc}{nt}")
                            nc.vector.tensor_tensor(
                                g3[:].rearrange("p (r c) -> p r c", c=64),
                                g1[:].rearrange("p (r c) -> p r c", c=64),
                                _pad_rhs(y2_pad[oc], nt, 0, 0).bitcast(f32), OP.add)
                            nc.sync.dma_start(
                                out_d[128 * oc:128 * oc + 128,
                                      512 * nt:512 * nt + 512],
                                g3[:])
                        _conv_block(nc, psA, f"mc{oc}", mwts, y2_pad, 2, evac_mlp)

    nc.compile()
    return nc


def _prep(inputs):
    def bn_fold(g, b, m, v):
        s = g.astype(np.float64) / np.sqrt(v.astype(np.float64) + EPS)
        return s, b.astype(np.float64) - m.astype(np.float64) * s

    scale = C ** (-0.5)
    s_qkv, b_qkv = bn_fold(inputs['qkv_g'], inputs['qkv_b'], inputs['qkv_m'], inputs['qkv_v'])
    qkv_w = inputs['qkv_w'].astype(np.float64)
    # [j, tap, i, o]
    qkv_wT = (qkv_w * s_qkv[:, :, None, None, None]).transpose(0, 3, 4, 2, 1).reshape(3, 9, C, C)
    qkvw = qkv_wT.reshape(3, 9, 2, 128, 2, 128).transpose(0, 1, 2, 4, 3, 5).astype(np.float32)
    qkvw = np.ascontiguousarray(qkvw)  # [3, 9, ic, oc, 128, 128]

    s_ds, b_ds = bn_fold(inputs['ds_g'], inputs['ds_b'], inputs['ds_m'], inputs['ds_v'])
    pw = inputs['pw_w'].astype(np.float64)[:, :, :, 0, 0]              # [4, o, i]
    dw = inputs['dw_w'].astype(np.float64)[:, :, 0, :, :].reshape(4, C, 9)  # [4, i, tap]
    # dsT[xx, tap, i, o] = pw[xx,o,i] * dw[xx,i,tap] * s_ds[xx,o] * scale
    dsT = (pw.transpose(0, 2, 1)[:, None, :, :] * dw.transpose(0, 2, 1)[:, :, :, None]
           * s_ds[:, None, None, :]) * scale
    dsw = np.ascontiguousarray(dsT.reshape(4, 9, 2, 128, C).astype(np.float32))

    s_f, b_f = bn_fold(inputs['fuse_g'], inputs['fuse_b'], inputs['fuse_m'], inputs['fuse_v'])
    fuse_wT = (inputs['fuse_w'].astype(np.float64) * s_f[:, None, None, None]
               ).transpose(2, 3, 1, 0).reshape(9, 2 * C, C)
    fusew = np.ascontiguousarray(
        fuse_wT.reshape(9, 4, 128, 2, 128).transpose(0, 1, 3, 2, 4).astype(np.float32))

    s_n, t_n = bn_fold(inputs['norm_g'], inputs['norm_b'], inputs['norm_m'], inputs['norm_v'])
    s_m, b_m = bn_fold(inputs['mlp_g'], inputs['mlp_b'], inputs['mlp_m'], inputs['mlp_v'])
    mlp_wT = (inputs['mlp_w'].astype(np.float64) * s_m[:, None, None, None]
              ).transpose(2, 3, 1, 0).reshape(9, C, C)
    mlpw = np.ascontiguousarray(
        mlp_wT.reshape(9, 2, 128, 2, 128).transpose(0, 1, 3, 2, 4).astype(np.float32))

    consts = np.zeros((2, 128, 16), np.float64)
    cols = [b_qkv[0], b_qkv[1], b_qkv[2],
            b_ds[0] + b_ds[1], b_ds[2] + b_ds[3],
            b_f, s_n, t_n, b_m]
    for ci, v in enumerate(cols):
        consts[0, :, ci] = v[0:128]
        consts[1, :, ci] = v[128:256]
    consts = consts.astype(np.float32)

    ident = np.eye(128, dtype=np.float32)
    return qkvw, dsw, fusew, mlpw, consts, ident


def _host_pad(xb):
    """[C, H, W] -> [2, 128, PADLEN] zero-padded layout."""
    xp = np.zeros((2, 128, PADLEN), np.float32)
    xp[:, :, 68:68 + 64 * 66].reshape(2, 128, 64, 66)[:, :, :, 0:64] = \
        xb.reshape(2, 128, H, W)
    return xp


def kernel(**inputs):
    inputs = {k: np.asarray(v) for k, v in inputs.items()}
    if "nc" not in _CACHE:
        _CACHE["nc"] = _build()
    nc = _CACHE["nc"]

    qkvw, dsw, fusew, mlpw, consts, ident = _prep(inputs)
    shared = {"qkvw": qkvw, "dsw": dsw, "fusew": fusew, "mlpw": mlpw,
              "consts": consts, "ident": ident}
    x = inputs['x'].astype(np.float32)
    in_maps = [{"xp": _host_pad(x[b]), **shared} for b in range(B)]

    res = run_bass_kernel_spmd(nc, in_maps, core_ids=list(range(8)))
    out = np.stack([res.results[b]["out"] for b in range(B)])
    return out.reshape(B, C, H, W).astype(np.float32)


# revision 5
# speedup vs baseline: 1.2072x; 1.1830x over previous
"""Trainium2 Bass kernel for nn_ADSA_31061203484966 (channel-attention dense
transformer block). Pure data-parallel over batch B=8 across 8 NeuronCores.

Self-contained: hardcodes shapes; host-side numpy folds BN into conv
weights, folds depthwise+pointwise+attention-scale into dense per-tap
matrices, and the device kernel does all convs as shifted f32r matmuls
over a zero-padded [C, 4360] layout.
"""
import sys

for _p in ("/opt/trn_rl_repo", "/root/.axon_site/_ro/trn_rl_repo"):
    if _p not in sys.path:
        sys.path.append(_p)

import numpy as np
import concourse.bass as bass
import concourse.tile as tile
from concourse import bacc, mybir
from concourse.bass_utils import run_bass_kernel_spmd

f32 = mybir.dt.float32
f32r = mybir.dt.float32r
AF = mybir.ActivationFunctionType
OP = mybir.AluOpType

B, C, H, W = 8, 256, 64, 64
NH, HD = 4, 64
N = H * W                    # 4096
EPS = 1e-5
PADLEN = 4360                # 66*66 guarded padded row-major layout (+4 slack)
# image pixel (r, c) lives at column 68 + 66*r + c
TAPS = [(ky - 1, kx - 1) for ky in range(3) for kx in range(3)]

_CACHE = {}


def _pad_off(row, dx=0):
    return 68 + 66 * row + dx


def _pad_dst(tl, nt):
    """Strided write AP covering compact rows [8nt, 8nt+8) of a padded tile."""
    off = _pad_off(8 * nt)
    return tl[:, off:off + 528].rearrange("p (r c) -> p r c", c=66)[:, :, 0:64]


def _pad_rhs(tl, nt, dy, dx):
    """Conv rhs AP: 8 rows x 64 cols shifted by tap (dy, dx)."""
    off = _pad_off(8 * nt + dy, dx)
    return tl[:, off:off + 528].rearrange("p (r c) -> p r c", c=66)[:, :, 0:64]


def _zero_pads(nc, tl, zeros):
    """Zero the pad region of a [128, PADLEN] tile (dtype-safe via copies)."""
    nc.vector.tensor_copy(tl[:, 0:68], zeros[:, 0:68])
    nc.vector.tensor_copy(
        tl[:, 132:132 + 64 * 66].rearrange("p (r c) -> p r c", c=66)[:, :, 0:2],
        zeros[:, 0:128].rearrange("p (r c) -> p r c", c=2))
    nc.vector.tensor_copy(tl[:, 4292:4360], zeros[:, 0:68])


def _conv_block(nc, psA, name, wts, srcs, n_in, evac):
    """Weight-stationary grouped conv: for each 4-tile block, loop weights
    outer and 4 output tiles inner so consecutive matmuls share lhsT.

    wts: dict (t, ic) -> weight AP; srcs: list of padded input tiles
    (len n_in); evac(nt, ps) emits the PSUM evacuation."""
    n_mm = 9 * n_in
    for ntb in range(2):
        pss = [psA.tile([128, 512], f32, tag="conv", name=f"{name}ps{ntb}_{i}")
               for i in range(4)]
        mm = 0
        for ic in range(n_in):
            for t, (dy, dx) in enumerate(TAPS):
                w_ = wts[(t, ic)]
                for ntl in range(4):
                    nt = 4 * ntb + ntl
                    nc.tensor.matmul(
                        pss[ntl][:], w_[:], _pad_rhs(srcs[ic], nt, dy, dx),
                        start=(mm == 0), stop=(mm == n_mm - 1))
                mm += 1
        for ntl in range(4):
            evac(4 * ntb + ntl, pss[ntl])


def _build():
    nc = bacc.Bacc("TRN2", target_bir_lowering=False, debug=False, num_devices=8)

    xp_d = nc.dram_tensor("xp", [2, 128, PADLEN], f32, kind="ExternalInput").ap()
    qkvw_d = nc.dram_tensor("qkvw", [3, 9, 2, 2, 128, 128], f32, kind="ExternalInput").ap()
    dsw_d = nc.dram_tensor("dsw", [4, 9, 2, 128, 256], f32, kind="ExternalInput").ap()
    fusew_d = nc.dram_tensor("fusew", [9, 4, 2, 128, 128], f32, kind="ExternalInput").ap()
    mlpw_d = nc.dram_tensor("mlpw", [9, 2, 2, 128, 128], f32, kind="ExternalInput").ap()
    consts_d = nc.dram_tensor("consts", [2, 128, 16], f32, kind="ExternalInput").ap()
    ident_d = nc.dram_tensor("ident", [128, 128], f32, kind="ExternalInput").ap()
    out_d = nc.dram_tensor("out", [C, N], f32, kind="ExternalOutput").ap()

    with tile.TileContext(nc) as tc:
        with tc.tile_pool(name="persist", bufs=1) as persist, \
             tc.tile_pool(name="psA", bufs=4, space="PSUM") as psA, \
             tc.tile_pool(name="psT", bufs=2, space="PSUM") as psT, \
             tc.tile_pool(name="psB", bufs=2, space="PSUM") as psB:

            zeros = persist.tile([128, 128], f32, name="zeros")
            nc.vector.memset(zeros[:], 0.0)
            ones = persist.tile([128, 128], f32, name="ones")
            nc.vector.memset(ones[:], 1.0)
            ident = persist.tile([128, 128], f32, name="ident")
            nc.sync.dma_start(ident[:], ident_d[:])
            consts = [persist.tile([128, 16], f32, name=f"consts{m}") for m in range(2)]
            for m in range(2):
                nc.sync.dma_start(consts[m][:], consts_d[m])

            v_pad = [persist.tile([128, PADLEN], f32r, name=f"v_pad{m}") for m in range(2)]
            for m in range(2):
                _zero_pads(nc, v_pad[m], zeros)

            # blockdiag attention-weight chunks, one per (variant, chunk)
            wblk = {}
            for xx in range(4):
                for ch in range(2):
                    t_ = persist.tile([128, 128], f32r, name=f"wblk{xx}_{ch}")
                    nc.vector.tensor_copy(t_[:], zeros[:])
                    wblk[(xx, ch)] = t_
            ab_sb = [persist.tile([128, 65], f32, name=f"ab{i}") for i in range(8)]
            tmp_wt = [persist.tile([128, 64], f32, name=f"tmpwt{h}") for h in range(4)]
            nm_rq = [persist.tile([128, 1], f32, name=f"nmrq{h}") for h in range(4)]

            with tc.tile_pool(name="pqt", bufs=1) as pqt:
                bf16 = mybir.dt.bfloat16
                qT = pqt.tile([128, 32 * 260 + 64], bf16, name="qT")
                kT = pqt.tile([128, 32 * 260 + 64], bf16, name="kT")
                # ones columns (65th col of each head slot in every chunk)
                for T_all in (qT, kT):
                    nc.vector.tensor_copy(
                        T_all[:, 64:64 + 65 * 128].rearrange(
                            "p (a b) -> p a b", b=65)[:, :, 0:1],
                        ones[:, 0:128].rearrange("p (a b) -> p a b", b=1))

                with tc.tile_pool(name="px", bufs=1) as px, \
                     tc.tile_pool(name="wq", bufs=2) as wq, \
                     tc.tile_pool(name="stq", bufs=4) as stq:
                    x_pad = [px.tile([128, PADLEN], f32r, name=f"x_pad{m}") for m in range(2)]
                    for m in range(2):
                        nc.sync.dma_start(x_pad[m][:], xp_d[m].bitcast(f32r))

                    # ---- qkv convs (j: 0=q, 1=k, 2=v) ----
                    for j in range(3):
                        for m in range(2):
                            wts = {}
                            for t in range(9):
                                for ic in range(2):
                                    w_ = wq.tile([128, 128], f32r, tag=f"w{t}_{ic}",
                                                 name=f"qkvw{j}{m}{t}{ic}")
                                    nc.sync.dma_start(w_[:], qkvw_d[j, t, ic, m].bitcast(f32r))
                                    wts[(t, ic)] = w_

                            if j == 2:       # v -> padded buffer directly
                                def evac_v(nt, ps, m=m):
                                    nc.scalar.activation(
                                        _pad_dst(v_pad[m], nt),
                                        ps[:].rearrange("p (r c) -> p r c", c=64),
                                        AF.Identity, bias=consts[m][:, 2:3], scale=1.0)
                                _conv_block(nc, psA, f"v{m}", wts, x_pad, 2, evac_v)
                            else:            # q, k -> stage -> transpose -> qT/kT
                                T_all = qT if j == 0 else kT

                                def evac_qk(nt, ps, j=j, m=m, T_all=T_all):
                                    stg = stq.tile([128, 512], f32, tag="stage",
                                                   name=f"stg{j}{m}{nt}")
                                    nc.scalar.activation(
                                        stg[:], ps[:], AF.Identity,
                                        bias=consts[m][:, j:j + 1], scale=1.0)
                                    for bb in range(4):
                                        pst = psT.tile([128, 128], f32, tag="tr",
                                                       name=f"tr{j}{m}{nt}{bb}")
                                        nc.tensor.transpose(
                                            pst[:], stg[:, 128 * bb:128 * bb + 128],
                                            ident[:])
                                        base = 260 * (4 * nt + bb) + 130 * m
                                        nc.vector.tensor_copy(
                                            T_all[:, base:base + 130].rearrange(
                                                "p (h d) -> p h d", d=65)[:, :, 0:64],
                                            pst[:].rearrange("p (h d) -> p h d", d=64))
                                _conv_block(nc, psA, f"qk{j}{m}", wts, x_pad, 2, evac_qk)

                # ---- margins + W formation, interleaved per head ----
                for h in range(4):
                    for ab in range(2):
                        lhsT_src, rhs_src = (kT, qT) if ab == 0 else (qT, kT)
                        ps = psA.tile([128, 512], f32, tag="conv", name=f"mg{h}{ab}")
                        for c in range(32):
                            base = 260 * c + 65 * h
                            nc.tensor.matmul(
                                ps[0:65, 0:65],
                                lhsT_src[:, base:base + 65],
                                rhs_src[:, base:base + 65],
                                start=(c == 0), stop=(c == 31))
                        dst = ab_sb[2 * h + ab]
                        nc.vector.tensor_copy(dst[0:65, :], ps[0:65, 0:65])

                    a_sb, b_sb = ab_sb[2 * h], ab_sb[2 * h + 1]
                    ch, p0 = h // 2, 64 * (h % 2)
                    sl = slice(p0, p0 + 64)
                    # W_aa = S  (from B)
                    nc.vector.tensor_copy(wblk[(0, ch)][sl, sl], b_sb[0:64, 0:64])
                    # W_ai = R_q - S
                    nc.vector.tensor_scalar(
                        wblk[(2, ch)][sl, sl], b_sb[0:64, 0:64],
                        -1.0, b_sb[0:64, 64:65], OP.mult, OP.add)
                    # W_ia^T = R_k - S^T (from A), then transpose
                    nc.vector.tensor_scalar(
                        tmp_wt[h][0:64, 0:64], a_sb[0:64, 0:64],
                        -1.0, a_sb[0:64, 64:65], OP.mult, OP.add)
                    psw = psT.tile([128, 128], f32, tag="tr", name=f"psw{h}")
                    nc.tensor.transpose(psw[0:64, 0:64], tmp_wt[h][0:64, 0:64],
                                        ident[0:64, 0:64])
                    nc.vector.tensor_copy(wblk[(3, ch)][sl, sl], psw[0:64, 0:64])
                    # W_ii = (N - R_q) - W_ia
                    nc.vector.tensor_scalar(
                        nm_rq[h][0:64, 0:1], b_sb[0:64, 64:65],
                        -1.0, float(N), OP.mult, OP.add)
                    nc.vector.tensor_scalar(
                        wblk[(1, ch)][sl, sl], psw[0:64, 0:64],
                        -1.0, nm_rq[h][0:64, 0:1], OP.mult, OP.add)

            with tc.tile_pool(name="pcat", bufs=1) as pcat:
                cat_pad = [pcat.tile([128, PADLEN], f32r, name=f"cat_pad{i}")
                           for i in range(4)]
                for i in range(4):
                    _zero_pads(nc, cat_pad[i], zeros)

                with tc.tile_pool(name="pcw", bufs=1) as pcw, \
                     tc.tile_pool(name="wd", bufs=4) as wd:
                    cw = pcw.tile([128, 9216], f32r, name="cw")
                    # CW[pair, t, ic] = sum_xx blockdiag(W_xx).T @ dsT[xx, t][ic]
                    for pair in range(2):
                        for t in range(9):
                            for ic in range(2):
                                ps = psB.tile([128, 256], f32, tag="mg",
                                              name=f"cwp{pair}{t}{ic}")
                                for xi in range(2):
                                    xx = 2 * pair + xi
                                    w_ = wd.tile([128, 256], f32r, tag="dsw",
                                                 name=f"dsw{xx}{t}{ic}")
                                    nc.sync.dma_start(w_[:], dsw_d[xx, t, ic].bitcast(f32r))
                                    nc.tensor.matmul(ps[:], wblk[(xx, ic)][:], w_[:],
                                                     start=(xi == 0), stop=(xi == 1))
                                col = ((pair * 9 + t) * 2 + ic) * 256
                                nc.vector.tensor_copy(cw[:, col:col + 256], ps[:])

                    # ---- cat convs over v_pad ----
                    for pair in range(2):
                        for oc in range(2):
                            cwts = {}
                            for t in range(9):
                                for ic in range(2):
                                    col = ((pair * 9 + t) * 2 + ic) * 256 + 128 * oc
                                    cwts[(t, ic)] = cw[:, col:col + 128]

                            def evac_cat(nt, ps, pair=pair, oc=oc):
                                nc.scalar.activation(
                                    _pad_dst(cat_pad[2 * pair + oc], nt),
                                    ps[:].rearrange("p (r c) -> p r c", c=64),
                                    AF.Identity,
                                    bias=consts[oc][:, 3 + pair:4 + pair], scale=1.0)
                            _conv_block(nc, psA, f"cc{pair}{oc}", cwts, v_pad, 2,
                                        evac_cat)

                with tc.tile_pool(name="py2", bufs=1) as py2, \
                     tc.tile_pool(name="wf", bufs=2) as wf, \
                     tc.tile_pool(name="stf", bufs=4) as stf:
                    y2_pad = [py2.tile([128, PADLEN], f32r, name=f"y2_pad{m}")
                              for m in range(2)]
                    for m in range(2):
                        _zero_pads(nc, y2_pad[m], zeros)

                    # ---- fuse conv + gelu + residual(v) + norm affine ----
                    for oc in range(2):
                        fwts = {}
                        for t in range(9):
                            for ic in range(4):
                                w_ = wf.tile([128, 128], f32r, tag=f"f{t}_{ic}",
                                             name=f"fw{oc}{t}{ic}")
                                nc.sync.dma_start(w_[:], fusew_d[t, ic, oc].bitcast(f32r))
                                fwts[(t, ic)] = w_

                        def evac_fuse(nt, ps, oc=oc):
                            g1 = stf.tile([128, 512], f32, tag="g1", name=f"g1{oc}{nt}")
                            nc.scalar.activation(g1[:], ps[:], AF.Gelu_apprx_tanh,
                                                 bias=consts[oc][:, 5:6], scale=1.0)
                            g2 = stf.tile([128, 512], f32, tag="g2", name=f"g2{oc}{nt}")
                            nc.vector.tensor_tensor(
                                g2[:].rearrange("p (r c) -> p r c", c=64),
                                g1[:].rearrange("p (r c) -> p r c", c=64),
                                _pad_rhs(v_pad[oc], nt, 0, 0).bitcast(f32), OP.add)
                            nc.vector.tensor_scalar(
                                _pad_dst(y2_pad[oc], nt),
                                g2[:].rearrange("p (r c) -> p r c", c=64),
                                consts[oc][:, 6:7], consts[oc][:, 7:8],
                                OP.mult, OP.add)
                        _conv_block(nc, psA, f"fc{oc}", fwts, cat_pad, 4, evac_fuse)

                    # ---- mlp conv + gelu + residual(y2) -> out ----
                    for oc in range(2):
                        mwts = {}
                        for t in range(9):
                            for ic in range(2):
                                w_ = wf.tile([128, 128], f32r, tag=f"f{t}_{ic}",
                                             name=f"mw{oc}{t}{ic}")
                                nc.sync.dma_start(w_[:], mlpw_d[t, ic, oc].bitcast(f32r))
                                mwts[(t, ic)] = w_

                        def evac_mlp(nt, ps, oc=oc):
                            g1 = stf.tile([128, 512], f32, tag="g1", name=f"mg1{oc}{nt}")
                            nc.scalar.activation(g1[:], ps[:], AF.Gelu_apprx_tanh,
                                                 bias=consts[oc][:, 8:9], scale=1.0)
                            g3 = stf.tile([128, 512], f32, tag="g2", name=f"mo{oc}{nt}")
                            nc.vector.tensor_tensor(
                                g3[:].rearrange("p (r c) -> p r c", c=64),
                                g1[:].rearrange("p (r c) -> p r c", c=64),
                                _pad_rhs(y2_pad[oc], nt, 0, 0).bitcast(f32), OP.add)
                            nc.sync.dma_start(
                                out_d[128 * oc:128 * oc + 128,
                                      512 * nt:512 * nt + 512],
                                g3[:])
                        _conv_block(nc, psA, f"mc{oc}", mwts, y2_pad, 2, evac_mlp)

    nc.compile()
    return nc


def _prep(inputs):
    def bn_fold(g, b, m, v):
        s = g.astype(np.float64) / np.sqrt(v.astype(np.float64) + EPS)
        return s, b.astype(np.float64) - m.astype(np.float64) * s

    scale = C ** (-0.5)
    s_qkv, b_qkv = bn_fold(inputs['qkv_g'], inputs['qkv_b'], inputs['qkv_m'], inputs['qkv_v'])
    qkv_w = inputs['qkv_w'].astype(np.float64)
    # [j, tap, i, o]
    qkv_wT = (qkv_w * s_qkv[:, :, None, None, None]).transpose(0, 3, 4, 2, 1).reshape(3, 9, C, C)
    qkvw = qkv_wT.reshape(3, 9, 2, 128, 2, 128).transpose(0, 1, 2, 4, 3, 5).astype(np.float32)
    qkvw = np.ascontiguousarray(qkvw)  # [3, 9, ic, oc, 128, 128]

    s_ds, b_ds = bn_fold(inputs['ds_g'], inputs['ds_b'], inputs['ds_m'], inputs['ds_v'])
    pw = inputs['pw_w'].astype(np.float64)[:, :, :, 0, 0]              # [4, o, i]
    dw = inputs['dw_w'].astype(np.float64)[:, :, 0, :, :].reshape(4, C, 9)  # [4, i, tap]
    # dsT[xx, tap, i, o] = pw[xx,o,i] * dw[xx,i,tap] * s_ds[xx,o] * scale
    dsT = (pw.transpose(0, 2, 1)[:, None, :, :] * dw.transpose(0, 2, 1)[:, :, :, None]
           * s_ds[:, None, None, :]) * scale
    dsw = np.ascontiguousarray(dsT.reshape(4, 9, 2, 128, C).astype(np.float32))

    s_f, b_f = bn_fold(inputs['fuse_g'], inputs['fuse_b'], inputs['fuse_m'], inputs['fuse_v'])
    fuse_wT = (inputs['fuse_w'].astype(np.float64) * s_f[:, None, None, None]
               ).transpose(2, 3, 1, 0).reshape(9, 2 * C, C)
    fusew = np.ascontiguousarray(
        fuse_wT.reshape(9, 4, 128, 2, 128).transpose(0, 1, 3, 2, 4).astype(np.float32))

    s_n, t_n = bn_fold(inputs['norm_g'], inputs['norm_b'], inputs['norm_m'], inputs['norm_v'])
    s_m, b_m = bn_fold(inputs['mlp_g'], inputs['mlp_b'], inputs['mlp_m'], inputs['mlp_v'])
    mlp_wT = (inputs['mlp_w'].astype(np.float64) * s_m[:, None, None, None]
              ).transpose(2, 3, 1, 0).reshape(9, C, C)
    mlpw = np.ascontiguousarray(
        mlp_wT.reshape(9, 2, 128, 2, 128).transpose(0, 1, 3, 2, 4).astype(np.float32))

    consts = np.zeros((2, 128, 16), np.float64)
    cols = [b_qkv[0], b_qkv[1], b_qkv[2],
            b_ds[0] + b_ds[1], b_ds[2] + b_ds[3],
            b_f, s_n, t_n, b_m]
    for ci, v in enumerate(cols):
        consts[0, :, ci] = v[0:128]
        consts[1, :, ci] = v[128:256]
    consts = consts.astype(np.float32)

    ident = np.eye(128, dtype=np.float32)
    return qkvw, dsw, fusew, mlpw, consts, ident


def _host_pad(xb):
    """[C, H, W] -> [2, 128, PADLEN] zero-padded layout."""
    xp = np.zeros((2, 128, PADLEN), np.float32)
    xp[:, :, 68:68 + 64 * 66].reshape(2, 128, 64, 66)[:, :, :, 0:64] = \
        xb.reshape(2, 128, H, W)
    return xp


def kernel(**inputs):
    inputs = {k: np.asarray(v) for k, v in inputs.items()}
    if "nc" not in _CACHE:
        _CACHE["nc"] = _build()
    nc = _CACHE["nc"]

    qkvw, dsw, fusew, mlpw, consts, ident = _prep(inputs)
    shared = {"qkvw": qkvw, "dsw": dsw, "fusew": fusew, "mlpw": mlpw,
              "consts": consts, "ident": ident}
    x = inputs['x'].astype(np.float32)
    in_maps = [{"xp": _host_pad(x[b]), **shared} for b in range(B)]

    res = run_bass_kernel_spmd(nc, in_maps, core_ids=list(range(8)))
    out = np.stack([res.results[b]["out"] for b in range(B)])
    return out.reshape(B, C, H, W).astype(np.float32)


# revision 7
# speedup vs baseline: 1.2184x; 1.0093x over previous
"""Trainium2 Bass kernel for nn_ADSA_31061203484966 (channel-attention dense
transformer block). Pure data-parallel over batch B=8 across 8 NeuronCores.

Self-contained: hardcodes shapes; host-side numpy folds BN into conv
weights, folds depthwise+pointwise+attention-scale into dense per-tap
matrices, and the device kernel does all convs as shifted f32r matmuls
over a zero-padded [C, 4360] layout.
"""
import sys

for _p in ("/opt/trn_rl_repo", "/root/.axon_site/_ro/trn_rl_repo"):
    if _p not in sys.path:
        sys.path.append(_p)

import numpy as np
import concourse.bass as bass
import concourse.tile as tile
from concourse import bacc, mybir
from concourse.bass_utils import run_bass_kernel_spmd

f32 = mybir.dt.float32
f32r = mybir.dt.float32r
AF = mybir.ActivationFunctionType
OP = mybir.AluOpType

B, C, H, W = 8, 256, 64, 64
NH, HD = 4, 64
N = H * W                    # 4096
EPS = 1e-5
PADLEN = 4360                # 66*66 guarded padded row-major layout (+4 slack)
PADHALF = 2248               # 34 padded rows + guards, per half (rows 0-33 / 32-65)
# image pixel (r, c) lives at column 68 + 66*r + c
TAPS = [(ky - 1, kx - 1) for ky in range(3) for kx in range(3)]

_CACHE = {}


def _pad_off(row, dx=0):
    return 68 + 66 * row + dx


def _pad_dst(tl, nt):
    """Strided write AP covering compact rows [8nt, 8nt+8) of a padded tile."""
    off = _pad_off(8 * nt)
    return tl[:, off:off + 528].rearrange("p (r c) -> p r c", c=66)[:, :, 0:64]


def _pad_rhs(tl, nt, dy, dx):
    """Conv rhs AP: 8 rows x 64 cols shifted by tap (dy, dx)."""
    off = _pad_off(8 * nt + dy, dx)
    return tl[:, off:off + 528].rearrange("p (r c) -> p r c", c=66)[:, :, 0:64]


def _zero_pads(nc, tl, zeros):
    """Zero the pad region of a [128, PADLEN] tile (dtype-safe via copies)."""
    nc.vector.tensor_copy(tl[:, 0:68], zeros[:, 0:68])
    nc.vector.tensor_copy(
        tl[:, 132:132 + 64 * 66].rearrange("p (r c) -> p r c", c=66)[:, :, 0:2],
        zeros[:, 0:128].rearrange("p (r c) -> p r c", c=2))
    nc.vector.tensor_copy(tl[:, 4292:4360], zeros[:, 0:68])


def _conv_block(nc, psA, name, wts, srcs, n_in, evac):
    """Weight-stationary grouped conv: for each 4-tile block, loop weights
    outer and 4 output tiles inner so consecutive matmuls share lhsT.

    wts: dict (t, ic) -> weight AP; srcs: list of padded input tiles
    (len n_in); evac(nt, ps) emits the PSUM evacuation."""
    n_mm = 9 * n_in
    for ntb in range(2):
        pss = [psA.tile([128, 512], f32, tag="conv", name=f"{name}ps{ntb}_{i}")
               for i in range(4)]
        mm = 0
        for ic in range(n_in):
            for t, (dy, dx) in enumerate(TAPS):
                w_ = wts[(t, ic)]
                for ntl in range(4):
                    nt = 4 * ntb + ntl
                    if callable(srcs):
                        tl, rshift = srcs(nt, ic)
                        off = _pad_off(8 * nt + dy, dx) - rshift
                        rhs = tl[:, off:off + 528].rearrange(
                            "p (r c) -> p r c", c=66)[:, :, 0:64]
                    else:
                        rhs = _pad_rhs(srcs[ic], nt, dy, dx)
                    nc.tensor.matmul(
                        pss[ntl][:], w_[:], rhs,
                        start=(mm == 0), stop=(mm == n_mm - 1))
                mm += 1
        for ntl in range(4):
            evac(4 * ntb + ntl, pss[ntl])


def _build():
    nc = bacc.Bacc("TRN2", target_bir_lowering=False, debug=False, num_devices=8)

    xp_d = nc.dram_tensor("xp", [2, 2, 128, PADHALF], f32, kind="ExternalInput").ap()
    qkvw_d = nc.dram_tensor("qkvw", [3, 9, 2, 2, 128, 128], f32, kind="ExternalInput").ap()
    dsw_d = nc.dram_tensor("dsw", [4, 9, 2, 128, 256], f32, kind="ExternalInput").ap()
    fusew_d = nc.dram_tensor("fusew", [9, 4, 2, 128, 128], f32, kind="ExternalInput").ap()
    mlpw_d = nc.dram_tensor("mlpw", [9, 2, 2, 128, 128], f32, kind="ExternalInput").ap()
    consts_d = nc.dram_tensor("consts", [2, 128, 16], f32, kind="ExternalInput").ap()
    ident_d = nc.dram_tensor("ident", [128, 128], f32, kind="ExternalInput").ap()
    out_d = nc.dram_tensor("out", [C, N], f32, kind="ExternalOutput").ap()

    with tile.TileContext(nc) as tc:
        with tc.tile_pool(name="persist", bufs=1) as persist, \
             tc.tile_pool(name="psA", bufs=4, space="PSUM") as psA, \
             tc.tile_pool(name="psT", bufs=2, space="PSUM") as psT, \
             tc.tile_pool(name="psB", bufs=2, space="PSUM") as psB:

            zeros = persist.tile([128, 128], f32, name="zeros")
            nc.vector.memset(zeros[:], 0.0)
            ones = persist.tile([128, 128], f32, name="ones")
            nc.vector.memset(ones[:], 1.0)
            ident = persist.tile([128, 128], f32, name="ident")
            nc.sync.dma_start(ident[:], ident_d[:])
            ident_bf = persist.tile([128, 128], mybir.dt.bfloat16, name="ident_bf")
            nc.vector.tensor_copy(ident_bf[:], ident[:])
            consts = [persist.tile([128, 16], f32, name=f"consts{m}") for m in range(2)]
            for m in range(2):
                nc.sync.dma_start(consts[m][:], consts_d[m])

            v_pad = [persist.tile([128, PADLEN], f32r, name=f"v_pad{m}") for m in range(2)]
            for m in range(2):
                _zero_pads(nc, v_pad[m], zeros)

            # blockdiag attention-weight chunks, one per (variant, chunk)
            wblk = {}
            for xx in range(4):
                for ch in range(2):
                    t_ = persist.tile([128, 128], f32r, name=f"wblk{xx}_{ch}")
                    nc.vector.tensor_copy(t_[:], zeros[:])
                    wblk[(xx, ch)] = t_
            ab_sb = [persist.tile([128, 65], f32, name=f"ab{i}") for i in range(8)]
            tmp_wt = [persist.tile([128, 64], f32, name=f"tmpwt{h}") for h in range(4)]
            nm_rq = [persist.tile([128, 1], f32, name=f"nmrq{h}") for h in range(4)]

            with tc.tile_pool(name="pqt", bufs=1) as pqt:
                bf16 = mybir.dt.bfloat16
                qT = pqt.tile([128, 32 * 260 + 64], bf16, name="qT")
                kT = pqt.tile([128, 32 * 260 + 64], bf16, name="kT")
                # ones columns (65th col of each head slot in every chunk)
                for T_all in (qT, kT):
                    nc.vector.tensor_copy(
                        T_all[:, 64:64 + 65 * 128].rearrange(
                            "p (a b) -> p a b", b=65)[:, :, 0:1],
                        ones[:, 0:128].rearrange("p (a b) -> p a b", b=1))

                with tc.tile_pool(name="px", bufs=1) as px, \
                     tc.tile_pool(name="wq", bufs=2) as wq, \
                     tc.tile_pool(name="stq", bufs=4) as stq:
                    x_pad = {}
                    for hh in range(2):
                        for m in range(2):
                            t_ = px.tile([128, PADHALF], f32r, name=f"x_pad{m}_{hh}")
                            nc.sync.dma_start(t_[:], xp_d[m, hh].bitcast(f32r))
                            x_pad[(m, hh)] = t_

                    def x_src(nt, ic):
                        hh = 0 if nt < 4 else 1
                        return x_pad[(ic, hh)], (32 * 66 if hh else 0)

                    # ---- qkv convs (j: 0=q, 1=k, 2=v) ----
                    for j in range(3):
                        for m in range(2):
                            wts = {}
                            for t in range(9):
                                for ic in range(2):
                                    w_ = wq.tile([128, 128], f32r, tag=f"w{t}_{ic}",
                                                 name=f"qkvw{j}{m}{t}{ic}")
                                    nc.sync.dma_start(w_[:], qkvw_d[j, t, ic, m].bitcast(f32r))
                                    wts[(t, ic)] = w_

                            if j == 2:       # v -> padded buffer directly
                                def evac_v(nt, ps, m=m):
                                    nc.scalar.activation(
                                        _pad_dst(v_pad[m], nt),
                                        ps[:].rearrange("p (r c) -> p r c", c=64),
                                        AF.Identity, bias=consts[m][:, 2:3], scale=1.0)
                                _conv_block(nc, psA, f"v{m}", wts, x_src, 2, evac_v)
                            else:            # q, k -> stage -> transpose -> qT/kT
                                T_all = qT if j == 0 else kT

                                def evac_qk(nt, ps, j=j, m=m, T_all=T_all):
                                    stg = stq.tile([128, 512], mybir.dt.bfloat16,
                                                   tag="stage", name=f"stg{j}{m}{nt}")
                                    nc.scalar.activation(
                                        stg[:], ps[:], AF.Identity,
                                        bias=consts[m][:, j:j + 1], scale=1.0)
                                    for bb in range(4):
                                        pst = psT.tile([128, 128], mybir.dt.bfloat16,
                                                       tag="tr", name=f"tr{j}{m}{nt}{bb}")
                                        nc.tensor.transpose(
                                            pst[:], stg[:, 128 * bb:128 * bb + 128],
                                            ident_bf[:])
                                        base = 260 * (4 * nt + bb) + 130 * m
                                        nc.vector.tensor_copy(
                                            T_all[:, base:base + 130].rearrange(
                                                "p (h d) -> p h d", d=65)[:, :, 0:64],
                                            pst[:].rearrange("p (h d) -> p h d", d=64))
                                _conv_block(nc, psA, f"qk{j}{m}", wts, x_src, 2, evac_qk)

                # ---- margins + W formation, interleaved per head ----
                for h in range(4):
                    for ab in range(2):
                        lhsT_src, rhs_src = (kT, qT) if ab == 0 else (qT, kT)
                        ps = psA.tile([128, 512], f32, tag="conv", name=f"mg{h}{ab}")
                        for c in range(32):
                            base = 260 * c + 65 * h
                            nc.tensor.matmul(
                                ps[0:65, 0:65],
                                lhsT_src[:, base:base + 65],
                                rhs_src[:, base:base + 65],
                                start=(c == 0), stop=(c == 31))
                        dst = ab_sb[2 * h + ab]
                        nc.vector.tensor_copy(dst[0:65, :], ps[0:65, 0:65])

                    a_sb, b_sb = ab_sb[2 * h], ab_sb[2 * h + 1]
                    ch, p0 = h // 2, 64 * (h % 2)
                    sl = slice(p0, p0 + 64)
                    # W_aa = S  (from B)
                    nc.vector.tensor_copy(wblk[(0, ch)][sl, sl], b_sb[0:64, 0:64])
                    # W_ai = R_q - S
                    nc.vector.tensor_scalar(
                        wblk[(2, ch)][sl, sl], b_sb[0:64, 0:64],
                        -1.0, b_sb[0:64, 64:65], OP.mult, OP.add)
                    # W_ia^T = R_k - S^T (from A), then transpose
                    nc.vector.tensor_scalar(
                        tmp_wt[h][0:64, 0:64], a_sb[0:64, 0:64],
                        -1.0, a_sb[0:64, 64:65], OP.mult, OP.add)
                    psw = psT.tile([128, 128], f32, tag="tr", name=f"psw{h}")
                    nc.tensor.transpose(psw[0:64, 0:64], tmp_wt[h][0:64, 0:64],
                                        ident[0:64, 0:64])
                    nc.vector.tensor_copy(wblk[(3, ch)][sl, sl], psw[0:64, 0:64])
                    # W_ii = (N - R_q) - W_ia
                    nc.vector.tensor_scalar(
                        nm_rq[h][0:64, 0:1], b_sb[0:64, 64:65],
                        -1.0, float(N), OP.mult, OP.add)
                    nc.vector.tensor_scalar(
                        wblk[(1, ch)][sl, sl], psw[0:64, 0:64],
                        -1.0, nm_rq[h][0:64, 0:1], OP.mult, OP.add)

            with tc.tile_pool(name="pcat", bufs=1) as pcat:
                cat_pad = [pcat.tile([128, PADLEN], f32r, name=f"cat_pad{i}")
                           for i in range(4)]
                for i in range(4):
                    _zero_pads(nc, cat_pad[i], zeros)

                with tc.tile_pool(name="pcw", bufs=1) as pcw, \
                     tc.tile_pool(name="wd", bufs=4) as wd:
                    cw = pcw.tile([128, 9216], f32r, name="cw")
                    # CW[pair, t, ic] = sum_xx blockdiag(W_xx).T @ dsT[xx, t][ic]
                    for pair in range(2):
                        for t in range(9):
                            for ic in range(2):
                                ps = psB.tile([128, 256], f32, tag="mg",
                                              name=f"cwp{pair}{t}{ic}")
                                for xi in range(2):
                                    xx = 2 * pair + xi
                                    w_ = wd.tile([128, 256], f32r, tag="dsw",
                                                 name=f"dsw{xx}{t}{ic}")
                                    nc.sync.dma_start(w_[:], dsw_d[xx, t, ic].bitcast(f32r))
                                    nc.tensor.matmul(ps[:], wblk[(xx, ic)][:], w_[:],
                                                     start=(xi == 0), stop=(xi == 1))
                                col = ((pair * 9 + t) * 2 + ic) * 256
                                nc.vector.tensor_copy(cw[:, col:col + 256], ps[:])

                    # ---- cat convs over v_pad ----
                    for pair in range(2):
                        for oc in range(2):
                            cwts = {}
                            for t in range(9):
                                for ic in range(2):
                                    col = ((pair * 9 + t) * 2 + ic) * 256 + 128 * oc
                                    cwts[(t, ic)] = cw[:, col:col + 128]

                            def evac_cat(nt, ps, pair=pair, oc=oc):
                                nc.scalar.activation(
                                    _pad_dst(cat_pad[2 * pair + oc], nt),
                                    ps[:].rearrange("p (r c) -> p r c", c=64),
                                    AF.Identity,
                                    bias=consts[oc][:, 3 + pair:4 + pair], scale=1.0)
                            _conv_block(nc, psA, f"cc{pair}{oc}", cwts, v_pad, 2,
                                        evac_cat)

                with tc.tile_pool(name="py2", bufs=1) as py2, \
                     tc.tile_pool(name="wf", bufs=2) as wf, \
                     tc.tile_pool(name="stf", bufs=4) as stf:
                    y2_pad = [py2.tile([128, PADLEN], f32r, name=f"y2_pad{m}")
                              for m in range(2)]
                    for m in range(2):
                        _zero_pads(nc, y2_pad[m], zeros)

                    # ---- fuse conv + gelu + residual(v) + norm affine ----
                    for oc in range(2):
                        fwts = {}
                        for t in range(9):
                            for ic in range(4):
                                w_ = wf.tile([128, 128], f32r, tag=f"f{t}_{ic}",
                                             name=f"fw{oc}{t}{ic}")
                                nc.sync.dma_start(w_[:], fusew_d[t, ic, oc].bitcast(f32r))
                                fwts[(t, ic)] = w_

                        def evac_fuse(nt, ps, oc=oc):
                            g1 = stf.tile([128, 512], f32, tag="g1", name=f"g1{oc}{nt}")
                            nc.scalar.activation(g1[:], ps[:], AF.Gelu_apprx_tanh,
                                                 bias=consts[oc][:, 5:6], scale=1.0)
                            g2 = stf.tile([128, 512], f32, tag="g2", name=f"g2{oc}{nt}")
                            nc.vector.tensor_tensor(
                                g2[:].rearrange("p (r c) -> p r c", c=64),
                                g1[:].rearrange("p (r c) -> p r c", c=64),
                                _pad_rhs(v_pad[oc], nt, 0, 0).bitcast(f32), OP.add)
                            nc.vector.tensor_scalar(
                                _pad_dst(y2_pad[oc], nt),
                                g2[:].rearrange("p (r c) -> p r c", c=64),
                                consts[oc][:, 6:7], consts[oc][:, 7:8],
                                OP.mult, OP.add)
                        _conv_block(nc, psA, f"fc{oc}", fwts, cat_pad, 4, evac_fuse)

                    # ---- mlp conv + gelu + residual(y2) -> out ----
                    for oc in range(2):
                        mwts = {}
                        for t in range(9):
                            for ic in range(2):
                                w_ = wf.tile([128, 128], f32r, tag=f"f{t}_{ic}",
                                             name=f"mw{oc}{t}{ic}")
                                nc.sync.dma_start(w_[:], mlpw_d[t, ic, oc].bitcast(f32r))
                                mwts[(t, ic)] = w_

                        def evac_mlp(nt, ps, oc=oc):
                            g1 = stf.tile([128, 512], f32, tag="g1", name=f"mg1{oc}{nt}")
                            nc.scalar.activation(g1[:], ps[:], AF.Gelu_apprx_tanh,
                                                 bias=consts[oc][:, 8:9], scale=1.0)
                            g3 = stf.tile([128, 512], f32, tag="g2", name=f"mo{oc}{nt}")
                            nc.vector.tensor_tensor(
                                g3[:].rearrange("p (r c) -> p r c", c=64),
                                g1[:].rearrange("p (r c) -> p r c", c=64),
                                _pad_rhs(y2_pad[oc], nt, 0, 0).bitcast(f32), OP.add)
                            nc.sync.dma_start(
                                out_d[128 * oc:128 * oc + 128,
                                      512 * nt:512 * nt + 512],
                                g3[:])
                        _conv_block(nc, psA, f"mc{oc}", mwts, y2_pad, 2, evac_mlp)

    nc.compile()
    return nc


def _prep(inputs):
    def bn_fold(g, b, m, v):
        s = g.astype(np.float64) / np.sqrt(v.astype(np.float64) + EPS)
        return s, b.astype(np.float64) - m.astype(np.float64) * s

    scale = C ** (-0.5)
    s_qkv, b_qkv = bn_fold(inputs['qkv_g'], inputs['qkv_b'], inputs['qkv_m'], inputs['qkv_v'])
    qkv_w = inputs['qkv_w'].astype(np.float64)
    # [j, tap, i, o]
    qkv_wT = (qkv_w * s_qkv[:, :, None, None, None]).transpose(0, 3, 4, 2, 1).reshape(3, 9, C, C)
    qkvw = qkv_wT.reshape(3, 9, 2, 128, 2, 128).transpose(0, 1, 2, 4, 3, 5).astype(np.float32)
    qkvw = np.ascontiguousarray(qkvw)  # [3, 9, ic, oc, 128, 128]

    s_ds, b_ds = bn_fold(inputs['ds_g'], inputs['ds_b'], inputs['ds_m'], inputs['ds_v'])
    pw = inputs['pw_w'].astype(np.float64)[:, :, :, 0, 0]              # [4, o, i]
    dw = inputs['dw_w'].astype(np.float64)[:, :, 0, :, :].reshape(4, C, 9)  # [4, i, tap]
    # dsT[xx, tap, i, o] = pw[xx,o,i] * dw[xx,i,tap] * s_ds[xx,o] * scale
    dsT = (pw.transpose(0, 2, 1)[:, None, :, :] * dw.transpose(0, 2, 1)[:, :, :, None]
           * s_ds[:, None, None, :]) * scale
    dsw = np.ascontiguousarray(dsT.reshape(4, 9, 2, 128, C).astype(np.float32))

    s_f, b_f = bn_fold(inputs['fuse_g'], inputs['fuse_b'], inputs['fuse_m'], inputs['fuse_v'])
    fuse_wT = (inputs['fuse_w'].astype(np.float64) * s_f[:, None, None, None]
               ).transpose(2, 3, 1, 0).reshape(9, 2 * C, C)
    fusew = np.ascontiguousarray(
        fuse_wT.reshape(9, 4, 128, 2, 128).transpose(0, 1, 3, 2, 4).astype(np.float32))

    s_n, t_n = bn_fold(inputs['norm_g'], inputs['norm_b'], inputs['norm_m'], inputs['norm_v'])
    s_m, b_m = bn_fold(inputs['mlp_g'], inputs['mlp_b'], inputs['mlp_m'], inputs['mlp_v'])
    mlp_wT = (inputs['mlp_w'].astype(np.float64) * s_m[:, None, None, None]
              ).transpose(2, 3, 1, 0).reshape(9, C, C)
    mlpw = np.ascontiguousarray(
        mlp_wT.reshape(9, 2, 128, 2, 128).transpose(0, 1, 3, 2, 4).astype(np.float32))

    consts = np.zeros((2, 128, 16), np.float64)
    cols = [b_qkv[0], b_qkv[1], b_qkv[2],
            b_ds[0] + b_ds[1], b_ds[2] + b_ds[3],
            b_f, s_n, t_n, b_m]
    for ci, v in enumerate(cols):
        consts[0, :, ci] = v[0:128]
        consts[1, :, ci] = v[128:256]
    consts = consts.astype(np.float32)

    ident = np.eye(128, dtype=np.float32)
    return qkvw, dsw, fusew, mlpw, consts, ident


def _host_pad(xb):
    """[C, H, W] -> [2, 2, 128, PADHALF] zero-padded halves (rows 0-33/32-65)."""
    xp = np.zeros((2, 128, PADLEN + 8), np.float32)
    xp[:, :, 68:68 + 64 * 66].reshape(2, 128, 64, 66)[:, :, :, 0:64] = \
        xb.reshape(2, 128, H, W)
    out = np.zeros((2, 2, 128, PADHALF), np.float32)
    out[:, 0] = xp[:, :, 0:PADHALF]
    out[:, 1] = xp[:, :, 32 * 66:32 * 66 + PADHALF]
    return out


def kernel(**inputs):
    inputs = {k: np.asarray(v) for k, v in inputs.items()}
    if "nc" not in _CACHE:
        _CACHE["nc"] = _build()
    nc = _CACHE["nc"]

    qkvw, dsw, fusew, mlpw, consts, ident = _prep(inputs)
    shared = {"qkvw": qkvw, "dsw": dsw, "fusew": fusew, "mlpw": mlpw,
              "consts": consts, "ident": ident}
    x = inputs['x'].astype(np.float32)
    in_maps = [{"xp": _host_pad(x[b]), **shared} for b in range(B)]

    res = run_bass_kernel_spmd(nc, in_maps, core_ids=list(range(8)))
    out = np.stack([res.results[b]["out"] for b in range(B)])
    return out.reshape(B, C, H, W).astype(np.float32)
